# revision 24
# baseline (speedup 1.0000x reference)
"""MedicalAttention multi-head attention kernel for 8 Trainium2 NeuronCores.

Problem (hardcoded): B=2, S=2048, E=768, H=12, D=64.
  out = softmax(Q K^T / sqrt(D) + mcw_h * mask_k) V, projected by Wo.

Sharding: 24 (batch, head) pairs -> core c handles batch c//4, heads
3*(c%4) .. 3*(c%4)+3.  Each core computes its 3 heads' projections,
attention, and a partial (transposed) output-projection contribution
[E, S]; the host sums the 4 partials per batch, transposes, adds bo.

Device-side layouts (per core):
  xqT/xkT/xvT [E, S]    transposed inputs (host-prepped)
  QT_h, KT_h  [64, S]   head projections, d on partitions
  V'_h        [S, 65]   natural V with a ones column (col 64) so the
                        PV matmul's col 64 accumulates softmax sums
  scoresT     [k, q]    chunks of [128, 2048] in PSUM; medical bias is
                        per-partition there -> folded into Exp's bias
  outT'_h     [65, S]   PV output; row 64 = softmax sums
  partialT    [E, S]    output-projection partial (pre-bias), DMA'd out

All matmuls run as float32r (fp32 data, fast PE path).
"""

import os
import sys

import numpy as np

B, S, E, H, D = 2, 2048, 768, 12, 64
HPC = 3          # heads per core
NCORES = 8
KC = E // 128    # 6 contraction chunks over E
MC = S // 128    # 16 chunks over S (128-partition chunks)
NQ = S // 512    # 4 free-dim chunks over S
SCALE = 1.0 / 8.0  # 1/sqrt(D)

_REPO_PATHS = ("/opt/trn_rl_repo",)


def _ensure_path():
    for p in _REPO_PATHS:
        if os.path.isdir(p) and p not in sys.path:
            sys.path.insert(0, p)


_BUILT = None


def _build_program():
    """Build the (SPMD, per-core) Bass program once."""
    global _BUILT
    if _BUILT is not None:
        return _BUILT
    _ensure_path()
    from contextlib import ExitStack

    import concourse.mybir as mybir
    import concourse.tile as tile
    from concourse import bacc
    from concourse.mybir import ActivationFunctionType as AFT

    f32 = mybir.dt.float32
    f32r = mybir.dt.float32r

    nc = bacc.Bacc("TRN2", target_bir_lowering=False, debug=False,
                   num_devices=NCORES)

    def din(name, shape, dt=f32):
        return nc.dram_tensor(name, list(shape), dt, kind="ExternalInput").ap()

    xqT = din("xqT", [E, S], f32r)
    xkT = din("xkT", [E, S], f32r)
    xvT = din("xvT", [E, S], f32r)
    wqT = din("wqT", [E, HPC * D], f32r)   # Wq[rows_h, :].T
    wkT = din("wkT", [E, HPC * D], f32r)
    wvT = din("wvT", [E, 256], f32r)       # Wv[rows_h, :].T zero-padded
    woT = din("woT", [HPC * D, E], f32r)   # Wo[:, cols_h].T
    ones_a = din("ones_a", [1, 64], f32r)
    ones_b = din("ones_b", [128, MC], f32r)
    bq = din("bqp", [128, 2])           # packed per-partition bias (col1 rows 0:64)
    bk = din("bkp", [128, 2])
    bv = din("bvr", [HPC * D])          # broadcast along partitions on DMA
    med = din("med", [128, MC, HPC])    # mcw_h * mask[b, m*128+p]

    outT = nc.dram_tensor("outT", [E, S], f32, kind="ExternalOutput").ap()

    with tile.TileContext(nc) as tc, ExitStack() as ctx:
        # ---- long-lived pools ------------------------------------------
        wpool = ctx.enter_context(tc.tile_pool(name="w", bufs=1))
        qkpool = ctx.enter_context(tc.tile_pool(name="qk", bufs=1))
        vpool = ctx.enter_context(tc.tile_pool(name="v", bufs=1))
        opool = ctx.enter_context(tc.tile_pool(name="o", bufs=1))
        gpool = ctx.enter_context(tc.tile_pool(name="g", bufs=2))

        # ---- load weights / small tensors ------------------------------
        wq_sb = wpool.tile([128, KC, HPC * D], f32r, tag="wq")
        wk_sb = wpool.tile([128, KC, HPC * D], f32r, tag="wk")
        wv_sb = wpool.tile([128, KC, 256], f32r, tag="wv")
        nc.sync.dma_start(out=wq_sb, in_=wqT.rearrange("(c p) m -> p c m", p=128))
        nc.sync.dma_start(out=wk_sb, in_=wkT.rearrange("(c p) m -> p c m", p=128))
        nc.sync.dma_start(out=wv_sb, in_=wvT.rearrange("(c p) m -> p c m", p=128))
        wo_a = wpool.tile([128, E], f32r, tag="woa")
        wo_b = wpool.tile([64, E], f32r, tag="wob")
        nc.sync.dma_start(out=wo_a, in_=woT[0:128, :])
        nc.sync.dma_start(out=wo_b, in_=woT[128:192, :])
        bq_sb = wpool.tile([128, 2], f32, tag="bq")
        bk_sb = wpool.tile([128, 2], f32, tag="bk")
        nc.sync.dma_start(out=bq_sb, in_=bq)
        nc.sync.dma_start(out=bk_sb, in_=bk)
        bv_sb = wpool.tile([128, HPC * D], f32, tag="bv")
        nc.sync.dma_start(out=bv_sb, in_=bv.partition_broadcast(128))
        med_sb = wpool.tile([128, MC, HPC], f32, tag="med")
        nc.sync.dma_start(out=med_sb, in_=med)
        ones_sb = wpool.tile([1, 64], f32r, tag="ones")
        nc.sync.dma_start(out=ones_sb, in_=ones_a)

        # QT/KT [d(h), s]: rows 0:64 = head0, 64:128 = head1 (pair tiles);
        # qkt2 packs head2's QT (rows 0:64) and KT (rows 64:128).
        qt01 = qkpool.tile([128, S], f32r, tag="qt01")
        kt01 = qkpool.tile([128, S], f32r, tag="kt01")
        qt2 = qkpool.tile([64, S], f32r, tag="qt2")
        kt2 = qkpool.tile([64, S], f32r, tag="kt2")

        def qt(h):
            return (qt01[0:64], qt01[64:128], qt2[0:64])[h]

        def kt(h):
            return (kt01[0:64], kt01[64:128], kt2[0:64])[h]

        vts = [vpool.tile([128, MC, 65], f32r, tag=f"vt{h}", name=f"vt{h}")
               for h in range(HPC)]

        ot01 = opool.tile([128, S], f32r, tag="ot01")
        ot2 = opool.tile([64, S], f32r, tag="ot2")

        def ot(h):
            return (ot01[0:64], ot01[64:128], ot2[0:64])[h]

        # ================= phase A: projections =========================
        with ExitStack() as ctxa:
            xpool = ctxa.enter_context(tc.tile_pool(name="x", bufs=3))
            ps_a = ctxa.enter_context(
                tc.tile_pool(name="psa", bufs=3, space="PSUM"))

            def load_xT(x_dram, pfx):
                chunks = []
                for c in range(KC):
                    xt = xpool.tile([128, S], f32r, tag=f"x{c % 3}",
                                    name=f"{pfx}x{c}")
                    nc.sync.dma_start(out=xt,
                                      in_=x_dram[c * 128:(c + 1) * 128, :])
                    chunks.append(xt)
                return chunks

            def qk_proj(x_chunks, w_sb, b_sb, dst_pair, dst2_rows, pfx):
                for mg, (m0, msz, dst, bcol) in enumerate((
                    (0, 128, dst_pair, 0),
                    (128, 64, dst2_rows, 1),
                )):
                    for n in range(NQ):
                        ps = ps_a.tile([128, 512], f32, tag="pp",
                                       name=f"{pfx}ps{mg}_{n}")
                        for c in range(KC):
                            nc.tensor.matmul(
                                ps[0:msz, :],
                                lhsT=w_sb[:, c, m0:m0 + msz],
                                rhs=x_chunks[c][:, n * 512:(n + 1) * 512],
                                start=(c == 0), stop=(c == KC - 1),
                            )
                        nc.vector.tensor_scalar_add(
                            dst[:, n * 512:(n + 1) * 512],
                            ps[0:msz, :],
                            b_sb[0:msz, bcol:bcol + 1],
                        )

            xq_chunks = load_xT(xqT, "q")
            qk_proj(xq_chunks, wq_sb, bq_sb, qt01, qt2[0:64], "q")
            xk_chunks = load_xT(xkT, "k")
            qk_proj(xk_chunks, wk_sb, bk_sb, kt01, kt2[0:64], "k")

            # V projection: V'_h [s, 65] (ones col 64)
            xv_chunks = load_xT(xvT, "v")
            for m in range(MC):
                ps = ps_a.tile([128, 256], f32, tag="pp", name=f"vps{m}")
                for c in range(KC):
                    nc.tensor.matmul(
                        ps,
                        lhsT=xv_chunks[c][:, m * 128:(m + 1) * 128],
                        rhs=wv_sb[:, c, :],
                        start=(c == 0), stop=(c == KC - 1),
                    )
                for h in range(HPC):
                    nc.vector.tensor_add(
                        vts[h][:, m, 0:64],
                        ps[:, h * 64:(h + 1) * 64],
                        bv_sb[:, h * 64:(h + 1) * 64],
                    )
            for h in range(HPC):
                nc.sync.dma_start(out=vts[h][:, :, 64:65], in_=ones_b)

        # ================= phase B: attention ===========================
        with ExitStack() as ctxb:
            epool = ctxb.enter_context(tc.tile_pool(name="e", bufs=2))
            spool = ctxb.enter_context(tc.tile_pool(name="s", bufs=2))
            rpool = ctxb.enter_context(tc.tile_pool(name="r", bufs=2))
            ps_s = ctxb.enter_context(
                tc.tile_pool(name="pss", bufs=1, space="PSUM"))
            ps_v = ctxb.enter_context(
                tc.tile_pool(name="psv", bufs=1, space="PSUM"))

            for h in range(HPC):
                pv_ps = ps_v.tile([65, S], f32, tag="pv", name=f"pv{h}")
                for m in range(MC):
                    sc_ps = ps_s.tile([128, S], f32, tag="sc",
                                      name=f"sc{h}_{m}")
                    for n in range(NQ):
                        nc.tensor.matmul(
                            sc_ps[:, n * 512:(n + 1) * 512],
                            lhsT=kt(h)[:, m * 128:(m + 1) * 128],
                            rhs=qt(h)[:, n * 512:(n + 1) * 512],
                            start=True, stop=True,
                        )
                    et = epool.tile([128, S], f32r, tag="exp", name=f"e{h}_{m}")
                    nc.scalar.activation(
                        et, sc_ps, AFT.Exp,
                        bias=med_sb[:, m, h:h + 1], scale=SCALE,
                    )
                    for n in range(NQ):
                        nc.tensor.matmul(
                            pv_ps[:, n * 512:(n + 1) * 512],
                            lhsT=vts[h][:, m, :],
                            rhs=et[:, n * 512:(n + 1) * 512],
                            start=(m == 0), stop=(m == MC - 1),
                        )
                # normalize: rows 0:64 / row 64.  Broadcast 1/row64 to 64
                # partitions via a PE outer product (ones[1,64]^T @ srow).
                srow = spool.tile([1, S], f32r, tag="srow", name=f"srow{h}")
                with nc.allow_low_precision(reason="f32r rounding of softmax "
                                            "denominators feeding a matmul"):
                    nc.vector.reciprocal(srow, pv_ps[64:65, :])
                r_ps = ps_s.tile([128, S], f32, tag="sc", name=f"rps{h}")
                for n in range(NQ):
                    nc.tensor.matmul(
                        r_ps[0:64, n * 512:(n + 1) * 512],
                        lhsT=ones_sb,
                        rhs=srow[:, n * 512:(n + 1) * 512],
                        start=True, stop=True,
                    )
                rbc = rpool.tile([64, S], f32, tag="rbc", name=f"rbc{h}")
                nc.vector.tensor_copy(rbc, r_ps[0:64, :])
                nc.vector.tensor_mul(ot(h), pv_ps[0:64, :], rbc)

        # ================= phase C: output projection ===================
        with ExitStack() as ctxc:
            ps_o = ctxc.enter_context(
                tc.tile_pool(name="pso", bufs=2, space="PSUM"))
            for me in range(KC):
                for n in range(NQ):
                    ps = ps_o.tile([128, 512], f32, tag="op", name=f"o{me}_{n}")
                    nc.tensor.matmul(
                        ps,
                        lhsT=wo_a[:, me * 128:(me + 1) * 128],
                        rhs=ot01[:, n * 512:(n + 1) * 512],
                        start=True, stop=False,
                    )
                    nc.tensor.matmul(
                        ps,
                        lhsT=wo_b[:, me * 128:(me + 1) * 128],
                        rhs=ot2[:, n * 512:(n + 1) * 512],
                        start=False, stop=True,
                    )
                    og = gpool.tile([128, 512], f32, tag="og",
                                    name=f"og{me}_{n}")
                    nc.vector.tensor_copy(og, ps)
                    nc.sync.dma_start(
                        out=outT[me * 128:(me + 1) * 128,
                                 n * 512:(n + 1) * 512],
                        in_=og,
                    )

    nc.compile()
    _BUILT = nc
    return nc


def _make_in_maps(inputs):
    """Host-side sharding: slice/transpose full inputs into per-core maps."""
    q = np.ascontiguousarray(inputs["query"], np.float32)
    k = np.ascontiguousarray(inputs["key"], np.float32)
    v = np.ascontiguousarray(inputs["value"], np.float32)
    mask = np.asarray(inputs["medical_mask"], np.float32)
    Wq = np.asarray(inputs["Wq"], np.float32)
    Wk = np.asarray(inputs["Wk"], np.float32)
    Wv = np.asarray(inputs["Wv"], np.float32)
    Wo = np.asarray(inputs["Wo"], np.float32)
    bq = np.asarray(inputs["bq"], np.float32)
    bk = np.asarray(inputs["bk"], np.float32)
    bv = np.asarray(inputs["bv"], np.float32)
    mcw = np.asarray(inputs["mcw"], np.float32).reshape(H)

    in_maps = []
    for c in range(NCORES):
        b = c // 4
        h0 = (c % 4) * HPC
        rows = slice(h0 * D, (h0 + HPC) * D)

        def pack_bias(bvec):
            p = np.zeros((128, 2), np.float32)
            p[:, 0] = bvec[rows][0:128]
            p[0:64, 1] = bvec[rows][128:192]
            return p

        wv_pad = np.zeros((E, 256), np.float32)
        wv_pad[:, 0:HPC * D] = Wv[rows, :].T

        med = np.empty((128, MC, HPC), np.float32)
        for h in range(HPC):
            med[:, :, h] = (mcw[h0 + h] * mask[b]).reshape(MC, 128).T

        in_maps.append({
            "xqT": np.ascontiguousarray(q[b].T),
            "xkT": np.ascontiguousarray(k[b].T),
            "xvT": np.ascontiguousarray(v[b].T),
            "wqT": np.ascontiguousarray(Wq[rows, :].T),
            "wkT": np.ascontiguousarray(Wk[rows, :].T),
            "wvT": wv_pad,
            "woT": np.ascontiguousarray(Wo[:, rows].T),
            "bqp": pack_bias(bq),
            "bkp": pack_bias(bk),
            "bvr": np.ascontiguousarray(bv[rows]),
            "med": med,
            "ones_a": np.ones((1, 64), np.float32),
            "ones_b": np.ones((128, MC), np.float32),
        })
    return in_maps


def _gather(results, inputs):
    bo = np.asarray(inputs["bo"], np.float32)
    out = np.empty((B, S, E), np.float32)
    for b in range(B):
        acc = np.zeros((E, S), np.float32)
        for c in range(b * 4, b * 4 + 4):
            acc += results[c]["outT"]
        out[b] = acc.T + bo
    return out


def kernel(trace=False, **inputs):
    nc = _build_program()
    in_maps = _make_in_maps(inputs)
    _ensure_path()
    from concourse.bass_utils import run_bass_kernel_spmd

    res = run_bass_kernel_spmd(nc, in_maps, list(range(NCORES)), trace=trace)
    out = _gather(res.results, inputs)
    if trace:
        kernel.last_exec_ns = res.exec_time_ns
        kernel.last_results = res
    return out


kernel.last_exec_ns = None
kernel.last_results = None
